# revision 24
# baseline (speedup 1.0000x reference)
"""BinaryLinear kernel for 8x TRN2 NeuronCores.

out = x @ (weight > 0)  with x [8192, 2048] f32, weight [2048, 2048] f32.

Sharding: data-parallel over batch (1024 rows/core), weight replicated.

Per core (M=1024, K=2048, N=2048). Schedule derived by iterating on
TimelineSim engine-occupancy traces (v1 baseline 168us sim / ~183us HW
-> 111us sim / ~107us HW loop-slope), with HW validation at each step.
At this point the kernel is near the wire floor: the 32MB/core of DMA
(24MB in + 8MB out) costs ~93us at the ~332GB/s effective per-core
rate, and the DMA engines run at ~84% occupancy over the span.

- Mixed precision split-K: k-tiles 0..9 in bf16; k-tiles 10..15 in
  fp8e4 via DoubleRow matmuls (2 k-tiles per pass, 2x PE rate, exact
  {0,1} weights; fp32 PSUM accumulation). Raises rel err from 1.7e-3
  to 1.65e-2 (verified bit-exact against the HW path on the reference
  inputs), under the 2e-2 gate, and cuts the PE matmul floor from
  109us to ~89us.
- Weight streams in three column tranches (1024 + 512 + 512 cols),
  k-tile-major within each, so all 16 k-tiles of the first tranche
  are resident early; later tranches arrive with slack while earlier
  groups execute. DVE binarizes to {0,1} (bf16 or fp8) per 512-col
  quarter.
- Ramp: while tranche-0 k-tiles arrive (~1.5us apart), matmuls run
  kt-OUTER across 4 live banks (bt0,bt1)x(nt0,nt1); bt2 joins at kt4
  (6 banks) and wraps around afterwards; x transposes fill PE slack.
- Steady state: nt-paired 2-bank groups, kt-inner, consecutive
  matmuls sharing the stationary xT tile (dedup halves LDWEIGHTS).
  Each pair carries the next bt's cast(DVE)/transpose(PE)/evict(ACT)
  chain interleaved so it never bursts at a pair boundary.
- All out-DMAs ride the SP HWDGE ring EMITTED AFTER every input DMA:
  ring FIFO order gives inputs strict wire priority (out traffic only
  flows once the ~70us input stream drains; a 20-deep SBUF staging
  pool absorbs the latency). No DMA-wait head-of-line-blocks any
  compute engine's stream.
- x transposed 128x128-blockwise on the PE (is_transpose vs identity,
  4 blocks per PSUM staging tile, one contiguous ACT eviction into a
  per-bt xT tile); the first two startup chunks transpose raw f32
  directly to skip the cast latency. The DMA XBAR transpose path is
  avoided (DMATranspose/DMACopy xbar-mode transitions serialize).
- Tail: the last bt's tranche-2/3 groups run unpaired and split into
  two half-width (256-col) accumulations, so earlier halves' eviction
  and out-DMA overlap later halves' matmuls and only one short chain
  (evict -> config -> 128KB transfer -> completion) trails the final
  matmul. Staging pools (xraw 4, wraw 5, osb 24) are sized so neither
  the input stream nor PSUM recycling ever stalls on buffer reuse.
"""

import numpy as np

import concourse.bass as bass
import concourse.mybir as mybir
import concourse.tile as tile
from concourse import bacc
from concourse.bass_utils import run_bass_kernel_spmd
from concourse.masks import make_identity

B, K, N = 8192, 2048, 2048
N_CORES = 8
MB = B // N_CORES          # 1024 batch rows per core
P = 128
KT = K // P                # 16 k-tiles
BT = MB // P               # 8 batch tiles per core
NT = 4                     # output column blocks of 512
NB = N // NT               # 512
HW = K // 2                # 1024

F32 = mybir.dt.float32
BF16 = mybir.dt.bfloat16
F8 = mybir.dt.float8e4

WAVE_JOIN_KT = 4           # kt at which bt2 joins the ramp wave

# k-tiles >= KT_F8 run in fp8e4 with DoubleRow packing (2 k-tiles per
# matmul at 2x rate). x quantization over those 768 columns raises the
# result's relative error from 1.7e-3 (pure bf16) to 1.65e-2 (verified
# bit-exact against the HW path), still under the 2e-2 gate; the
# binarized weight is exact in fp8.
KT_F8 = 10


def build_kernel(repeat: int = 1, mode: str = "full"):
    nc = bacc.Bacc(None, target_bir_lowering=False)
    x = nc.dram_tensor("x", [MB, K], F32, kind="ExternalInput")
    w = nc.dram_tensor("w", [K, N], F32, kind="ExternalInput")
    out = nc.dram_tensor("out", [MB, N], F32, kind="ExternalOutput")

    w3 = w[:].rearrange("(kt p) n -> p kt n", p=P)   # [128, 16, 2048]

    do_x = mode in ("full", "nomm", "xonly")
    do_w = mode in ("full", "nomm", "wonly")
    do_mm = mode in ("full", "mmonly")

    def body(tc, pools):
        (xraw_pool, xbf_pool, xT_pool, wraw_pool, wbin_pool,
         out_pool, psum_pool, tpsum_pool, const_pool) = pools

        ident = const_pool.tile([P, P], BF16, tag="ident", name="ident")
        make_identity(nc, ident)
        ident32 = const_pool.tile([P, P], F32, tag="ident32", name="ident32")
        make_identity(nc, ident32)

        xraw = {}   # (bt, half) -> [P, HW] f32
        xT = {}     # bt -> [P, K] bf16   (col = kt*P + m; kts < KT_F8)
        xT8 = {}    # bt -> [P, (KT-KT_F8)*P] fp8e4 (col = (kt-KT_F8)*P + m)
        wbin = {}   # (kt, nt) -> [P, NB] bf16      (kts < KT_F8)
        wbin8 = {}  # (ktp, nt) -> [P, 2*NB] fp8e4  (ktp in KT_F8//2..KT//2-1)
        evict_flip = [0]

        def _binarize(dst, src):
            nc.vector.tensor_scalar(out=dst, in0=src, scalar1=0.0,
                                    scalar2=None, op0=mybir.AluOpType.is_gt)

        def _wbin_dst(kt, nt):
            """Destination slice for a binarized [P, NB] w quarter."""
            if kt < KT_F8:
                wbin[kt, nt] = wbin_pool.tile(
                    [P, NB], BF16, tag=f"wbin_{kt}_{nt}", name=f"wb{kt}_{nt}")
                return wbin[kt, nt][:]
            ktp, j = divmod(kt, 2)
            if (ktp, nt) not in wbin8:
                wbin8[ktp, nt] = wbin_pool.tile(
                    [P, 2 * NB], F8, tag=f"wbin8_{ktp}_{nt}",
                    name=f"wb8{ktp}_{nt}")
            return wbin8[ktp, nt][:, j * NB:(j + 1) * NB]

        def dma_x(bt, half, quarters=False):
            if not do_x:
                return
            t = xraw_pool.tile([P, HW], F32, tag=f"xraw_{half}",
                               name=f"xr{bt}_{half}")
            h0 = half * HW
            if quarters:
                nc.sync.dma_start(t[:, :NB], x[bt * P:(bt + 1) * P,
                                               h0:h0 + NB])
                nc.sync.dma_start(t[:, NB:], x[bt * P:(bt + 1) * P,
                                               h0 + NB:h0 + HW])
            else:
                nc.sync.dma_start(t[:], x[bt * P:(bt + 1) * P, h0:h0 + HW])
            xraw[bt, half] = t

        def dma_w2(kt):
            """Tranche-0: [128,1024] w k-tile -> binarized quarters nt0,nt1."""
            dsts = [_wbin_dst(kt, j) for j in range(2)]
            if do_w:
                wr = wraw_pool.tile([P, HW], F32, tag="wraw2", name="wr")
                nc.sync.dma_start(wr[:], w3[:, kt, 0:HW])
                for j in range(2):
                    _binarize(dsts[j], wr[:, j * NB:(j + 1) * NB])
            else:
                for j in range(2):
                    nc.any.memset(dsts[j], 1.0)

        def dma_w1(kt, nt):
            """Tranche-2/3: [128,512] w k-tile quarter."""
            dst = _wbin_dst(kt, nt)
            if do_w:
                wr = wraw_pool.tile([P, NB], F32, tag="wraw1", name="wr")
                nc.sync.dma_start(wr[:], w3[:, kt, nt * NB:(nt + 1) * NB])
                _binarize(dst, wr[:])
            else:
                nc.any.memset(dst, 1.0)

        xbf = {}

        def cast_chunk(bt, ktg, on_dve=False):
            """Cast 512 cols of x(bt) f32 -> bf16 (ACT, or DVE for the
            steady-phase bts so the chain doesn't serialize behind ACT's
            PSUM evictions at pair boundaries)."""
            if bt not in xT:
                xT[bt] = xT_pool.tile([P, K], BF16, tag=f"xT_{bt}",
                                      name=f"xT_{bt}")
                xT8[bt] = xT_pool.tile([P, (KT - KT_F8) * P], F8, tag=f"xT8_{bt}",
                                       name=f"xT8_{bt}")
            if not do_x:
                if ktg == 0:
                    nc.any.memset(xT[bt][:], 1.0)
                    nc.any.memset(xT8[bt][:], 1.0)
                return
            half, off = divmod(ktg * 4 * P, HW)
            xb = xbf_pool.tile([P, 4 * P], BF16, tag=f"xbf_{ktg % 2}",
                               name=f"xbf{bt}_{ktg}")
            src = xraw[bt, half][:, off:off + 4 * P]
            if on_dve:
                nc.vector.tensor_copy(xb[:], src)
            else:
                nc.scalar.activation(xb[:], src,
                                     mybir.ActivationFunctionType.Copy)
            xbf[bt, ktg] = xb

        def transp_f32_chunk(bt, ktg):
            """Startup only: transpose 4 blocks straight from raw f32 x
            (skips the cast on the critical path); PSUM staging borrows a
            main-pool bank; ACT eviction converts f32 -> bf16."""
            if bt not in xT:
                xT[bt] = xT_pool.tile([P, K], BF16, tag=f"xT_{bt}",
                                      name=f"xT_{bt}")
                xT8[bt] = xT_pool.tile([P, (KT - KT_F8) * P], F8, tag=f"xT8_{bt}",
                                       name=f"xT8_{bt}")
            if not do_x:
                if ktg == 0:
                    nc.any.memset(xT[bt][:], 1.0)
                    nc.any.memset(xT8[bt][:], 1.0)
                return
            half, off = divmod(ktg * 4 * P, HW)
            tp = psum_pool.tile([P, 4 * P], F32, tag="ps", name="tpf")
            for i in range(4):
                nc.tensor.transpose(
                    tp[:, i * P:(i + 1) * P],
                    xraw[bt, half][:, off + i * P:off + (i + 1) * P],
                    ident32[:])
            nc.scalar.activation(
                xT[bt][:, ktg * 4 * P:(ktg + 1) * 4 * P], tp[:],
                mybir.ActivationFunctionType.Copy)

        def transp_chunk(bt, ktg):
            """PE: transpose 4 blocks into one PSUM staging tile; ACT
            evicts contiguously into xT[bt] / xT8[bt] (a chunk straddling
            the bf16/fp8 boundary gets one eviction per segment)."""
            if not do_x:
                return
            xb = xbf.pop((bt, ktg))
            tp = tpsum_pool.tile([P, 4 * P], BF16, tag="tps", name="tp")
            for i in range(4):
                nc.tensor.transpose(tp[:, i * P:(i + 1) * P],
                                    xb[:, i * P:(i + 1) * P], ident[:])
            k0 = ktg * 4
            nbf = min(max(KT_F8 - k0, 0), 4)       # leading bf16 kts
            if nbf:
                nc.scalar.activation(
                    xT[bt][:, k0 * P:(k0 + nbf) * P], tp[:, :nbf * P],
                    mybir.ActivationFunctionType.Copy)
            if nbf < 4:
                f0 = (k0 + nbf - KT_F8) * P
                nc.scalar.activation(
                    xT8[bt][:, f0:f0 + (4 - nbf) * P], tp[:, nbf * P:],
                    mybir.ActivationFunctionType.Copy)

        def cast_T(bt, ktg):
            cast_chunk(bt, ktg)
            transp_chunk(bt, ktg)

        def mm(ps, bt, nt, kt, start, stop):
            nc.tensor.matmul(
                ps[:], xT[bt][:, kt * P:(kt + 1) * P], wbin[kt, nt][:],
                start=start, stop=stop)

        def mm_dr(ps, bt, nt, ktp, stop):
            """fp8 DoubleRow matmul: contracts k-tiles 2*ktp and 2*ktp+1
            in one pass (2 fp8 weights per PE cell, 2x rate)."""
            a = (ktp - KT_F8 // 2) * 2 * P
            lhsT = xT8[bt][:, a:a + 2 * P].rearrange("p (j m) -> p j m", j=2)
            rhs = wbin8[ktp, nt][:].rearrange("p (j n) -> p j n", j=2)
            nc.tensor.matmul(ps[:], lhsT, rhs, start=False, stop=stop,
                             perf_mode=mybir.MatmulPerfMode.DoubleRow)

        def evict_out(ps, bt, nt, split=1, ring=None):
            """PSUM -> SBUF staging (ACT/DVE alternate) -> DRAM via the SP
            HWDGE ring. All out-DMAs are emitted after every input DMA, so
            the ring's FIFO order gives inputs strict wire priority: out
            traffic only flows once the input stream has drained (~t=70us),
            which is exactly when the wire goes idle. The staging pool is
            deep enough (20) to hold every eviction until then."""
            ring = ring or nc.sync
            c = NB // split
            for s in range(split):
                ot = out_pool.tile([P, c], F32, tag="osb", name="ot")
                evict_flip[0] ^= 1
                if evict_flip[0] == 0:
                    nc.vector.tensor_copy(ot[:], ps[:, s * c:(s + 1) * c])
                else:
                    nc.scalar.activation(ot[:], ps[:, s * c:(s + 1) * c],
                                         mybir.ActivationFunctionType.Copy)
                ring.dma_start(
                    out[bt * P:(bt + 1) * P,
                        nt * NB + s * c:nt * NB + (s + 1) * c], ot[:])

        def group(bt, nt, ring=None, split=1):
            """Single-nt group; split=2 runs two half-width (256-col)
            accumulations so the first half's eviction + out-DMA overlap
            the second half's matmuls (shrinks the kernel tail)."""
            ring = ring or nc.sync
            wd = NB // split
            for h in range(split):
                ps = psum_pool.tile([P, wd], F32, tag="ps", name="ps")
                if do_mm:
                    for kt in range(KT_F8):
                        nc.tensor.matmul(
                            ps[:], xT[bt][:, kt * P:(kt + 1) * P],
                            wbin[kt, nt][:, h * wd:(h + 1) * wd],
                            start=(kt == 0), stop=False)
                    for ktp in range(KT_F8 // 2, KT // 2):
                        a = (ktp - KT_F8 // 2) * 2 * P
                        lhsT = xT8[bt][:, a:a + 2 * P].rearrange(
                            "p (j m) -> p j m", j=2)
                        rhs = wbin8[ktp, nt][:].rearrange(
                            "p (j n) -> p j n", j=2)[:, :, h * wd:(h + 1) * wd]
                        nc.tensor.matmul(
                            ps[:], lhsT, rhs, start=False,
                            stop=(ktp == KT // 2 - 1),
                            perf_mode=mybir.MatmulPerfMode.DoubleRow)
                else:
                    nc.any.memset(ps[:], 0.0)
                ot = out_pool.tile([P, wd], F32, tag="osb", name="ot")
                evict_flip[0] ^= 1
                if evict_flip[0] == 0:
                    nc.vector.tensor_copy(ot[:], ps[:])
                else:
                    nc.scalar.activation(ot[:], ps[:],
                                         mybir.ActivationFunctionType.Copy)
                ring.dma_start(
                    out[bt * P:(bt + 1) * P,
                        nt * NB + h * wd:nt * NB + (h + 1) * wd], ot[:])

        def group_pair(bt, nts, ring=None, prep=None):
            """Two nt blocks of one bt, kt-inner with consecutive matmuls
            sharing the stationary xT tile - halves the LDWEIGHTS count
            (the dedup pass strips the second load of each kt). k-tiles
            12..15 contract via two fp8 DoubleRow matmuls. `prep` names a
            bt whose cast/transpose chain is emitted interleaved with the
            matmuls (DVE cast, PE transpose, ACT evict all overlap the
            pair instead of bursting at its boundary)."""
            psa = psum_pool.tile([P, NB], F32, tag="ps", name="ps")
            psb = psum_pool.tile([P, NB], F32, tag="ps", name="ps")
            if do_mm:
                for kt in range(KT_F8):
                    mm(psa, bt, nts[0], kt, kt == 0, False)
                    mm(psb, bt, nts[1], kt, kt == 0, False)
                    if prep is not None and kt in (3, 5, 7, 9):
                        g = (kt - 3) // 2
                        cast_chunk(prep, g, on_dve=True)
                        transp_chunk(prep, g)
                for ktp in range(KT_F8 // 2, KT // 2):
                    last = ktp == KT // 2 - 1
                    mm_dr(psa, bt, nts[0], ktp, last)
                    mm_dr(psb, bt, nts[1], ktp, last)
            else:
                nc.any.memset(psa[:], 0.0)
                nc.any.memset(psb[:], 0.0)
                if prep is not None:
                    for g in range(4):
                        cast_T(prep, g)
            evict_out(psa, bt, nts[0], ring=ring)
            evict_out(psb, bt, nts[1], ring=ring)

        # ================= emission script =================
        # startup: first x chunks and w k-tiles interleaved so the first
        # wave matmul can issue as early as possible (w-kt0 split in two
        # so wbin[0,0] lands right as xT[0] k-tile 0 is ready); x h1
        # halves deferred behind the next w k-tile (their transposes are
        # only needed a few k-tiles later); x3's h1 comes after the fp8
        # k-tiles 12..15 so the DoubleRow tail isn't delayed
        dma_x(0, 0, quarters=True)
        dma_w1(0, 0)
        dma_w1(0, 1)
        dma_x(0, 1)
        dma_w1(1, 0)
        dma_w1(1, 1)
        dma_x(1, 0)
        dma_w1(2, 0)
        dma_w1(2, 1)
        dma_x(1, 1)
        dma_w1(3, 0)
        dma_w1(3, 1)
        dma_x(2, 0)
        dma_w1(4, 0)
        dma_w1(4, 1)
        dma_x(2, 1)
        for kt in range(5, KT):
            dma_w2(kt)
            if kt == 7:
                dma_x(3, 0)
        dma_x(3, 1)
        # rest of x right after tranche-0; then tranches 2 and 3
        for bt in range(4, BT):
            dma_x(bt, 0)
            dma_x(bt, 1)
        for kt in range(KT):
            dma_w1(kt, 2)
        for kt in range(KT):
            dma_w1(kt, 3)

        # transposes for bt0, bt1 up front (PE warms up on these; bt0's
        # first two chunks transpose raw f32 directly - no cast latency
        # on the critical startup path)
        transp_f32_chunk(0, 0)
        transp_f32_chunk(0, 1)
        cast_T(0, 2)
        cast_T(0, 3)
        for ktg in range(4):
            cast_T(1, ktg)

        # ramp wave: kt-outer, (bt0,bt1)x(nt0,nt1); bt2 joins at kt4 and
        # wraps around; transposes for bt2 fill PE slack, bt3's casts are
        # prefetched on ACT near the wave end (its x lands ~kt13)
        if do_mm:
            wave = [(0, 0), (0, 1), (1, 0), (1, 1)]
            pss = {g: psum_pool.tile([P, NB], F32, tag="ps", name=f"wps{g}")
                   for g in wave}
            join = [(2, 0), (2, 1)]
            for kt in range(KT_F8):
                for bt_, nt_ in wave:
                    mm(pss[bt_, nt_], bt_, nt_, kt, kt == 0, False)
                if kt == WAVE_JOIN_KT - 2:
                    cast_T(2, 0)
                    cast_T(2, 1)
                elif kt == WAVE_JOIN_KT - 1:
                    cast_T(2, 2)
                    cast_T(2, 3)
                    for g in join:
                        pss[g] = psum_pool.tile([P, NB], F32, tag="ps",
                                                name=f"wps{g}")
                if kt >= WAVE_JOIN_KT:
                    for bt_, nt_ in join:
                        mm(pss[bt_, nt_], bt_, nt_, kt,
                           kt == WAVE_JOIN_KT, False)
                # bt3's h0 cast/transpose prefetched as wave filler (its
                # x h0 lands around kt8 of tranche-0)
                if kt == 8:
                    cast_chunk(3, 0)
                elif kt == 9:
                    cast_chunk(3, 1)
                    transp_chunk(3, 0)

            # bt2 wraps kt0..3 while the fp8 k-tiles stream in, then the
            # fp8 DoubleRow tails close all six accumulations; bt3's h1
            # chunks (x3 h1 lands after the fp8 k-tiles) follow
            for kt in range(WAVE_JOIN_KT):
                for bt_, nt_ in join:
                    mm(pss[bt_, nt_], bt_, nt_, kt, False, False)
            transp_chunk(3, 1)
            cast_chunk(3, 2)
            cast_chunk(3, 3)
            for ktp in range(KT_F8 // 2, KT // 2):
                last = ktp == KT // 2 - 1
                for bt_, nt_ in wave + join:
                    mm_dr(pss[bt_, nt_], bt_, nt_, ktp, last)
            transp_chunk(3, 2)
            transp_chunk(3, 3)
            evict_flip[0] = 1
            for g in wave + join:
                evict_out(pss[g], g[0], g[1])
        else:
            for bt_ in (2, 3):
                for g in range(4):
                    cast_T(bt_, g)

        # steady state: tranche-0 nt-paired groups for bt3..7, each pair
        # carrying the next bt's cast/transpose chain interleaved
        for bt in range(3, BT):
            group_pair(bt, (0, 1), prep=bt + 1 if bt + 1 < BT else None)

        # tranche-2/3 nt-paired groups; the last bt runs as two single
        # groups so only one out-DMA remains after the final matmul
        for bt in range(BT - 1):
            group_pair(bt, (2, 3))
        group(BT - 1, 2, split=2)
        group(BT - 1, 3, split=2)

    with tile.TileContext(nc) as tc:
        with (
            tc.tile_pool(name="xraw", bufs=4) as xraw_pool,
            tc.tile_pool(name="xbf", bufs=2) as xbf_pool,
            tc.tile_pool(name="xT", bufs=1) as xT_pool,
            tc.tile_pool(name="wraw", bufs=5) as wraw_pool,
            tc.tile_pool(name="wbin", bufs=1) as wbin_pool,
            tc.tile_pool(name="osb", bufs=24) as out_pool,
            tc.tile_pool(name="ps", bufs=6, space="PSUM") as psum_pool,
            tc.tile_pool(name="tps", bufs=2, space="PSUM") as tpsum_pool,
            tc.tile_pool(name="const", bufs=1) as const_pool,
        ):
            pools = (xraw_pool, xbf_pool, xT_pool, wraw_pool, wbin_pool,
                     out_pool, psum_pool, tpsum_pool, const_pool)
            if repeat == 1:
                body(tc, pools)
            else:
                with tc.For_i(0, repeat, 1):
                    body(tc, pools)
    _dedup_ldweights(nc)
    nc.compile()
    return nc


def _ldw_key(ins):
    ap = ins.ins[0]
    bap = getattr(ap, "bass_ap", None)
    return (getattr(ap, "memref", None), getattr(bap, "offset", None),
            str(getattr(bap, "ap", None)), getattr(ins, "is_transpose", None))


def _dedup_ldweights(nc):
    """Remove PE weight reloads of the already-loaded stationary operand."""
    removed = 0
    for bb in nc.main_func.blocks:
        il = bb.instructions
        last_key = None
        drop = []
        for idx, ins in enumerate(il):
            if not isinstance(ins, mybir.InstLdweights):
                continue
            si = ins.sync_info
            has_sync = si is not None and (
                (si.on_wait and len(si.on_wait) > 0)
                or (si.on_update and len(si.on_update) > 0))
            key = _ldw_key(ins)
            if key == last_key and not has_sync:
                drop.append(idx)
                removed += 1
            else:
                last_key = key
        for idx in reversed(drop):
            del il[idx]
    return removed


_NC_CACHE = None


def _get_nc():
    global _NC_CACHE
    if _NC_CACHE is None:
        _NC_CACHE = build_kernel()
    return _NC_CACHE


def kernel(x: np.ndarray, weight: np.ndarray):
    assert x.shape == (B, K) and weight.shape == (K, N)
    x = np.ascontiguousarray(x, dtype=np.float32)
    weight = np.ascontiguousarray(weight, dtype=np.float32)
    nc = _get_nc()
    in_maps = [
        {"x": x[i * MB:(i + 1) * MB], "w": weight}
        for i in range(N_CORES)
    ]
    res = run_bass_kernel_spmd(nc, in_maps, core_ids=list(range(N_CORES)))
    return np.concatenate([res.results[i]["out"] for i in range(N_CORES)], axis=0)
